# revision 25
# baseline (speedup 1.0000x reference)
"""Trainium2 Bass kernel for nn_CMDPEncoder (VQ codebook quantize + random
batch-mix dequantize + DP noise).

Reference semantics:
    dots = einsum('bsd,vd->bsv', base, codebook)
    qi   = argmin_v(csq[v] - 2*dots)                  # [B,S]
    codes[b,s,j] = qi[rand_idx[b,s,j], s]
    out  = mean_j codebook[codes] + 0.1*noise

Sharding: split the sequence dim S across the 8 cores (64 positions each).
The rand_idx mixing crosses only the batch dim at fixed s, so with S-sharding
every core's mixing is fully local (no collectives).  Tokens are laid out
s-major (t = s_local*16 + b) so each 128-token tile holds 8 complete
s-groups of 16 batches, and the mix becomes a block-diagonal [128,128]
matmul with host-precomputed weights (counts/4 from rand_idx).

v2 design (vs the fp32/bf16 baseline):
  - All scoring in fp16: x and codebook streamed as fp16 (same 1 cyc/row PE
    speed as bf16, 8x the mantissa), scores kept as fp16 in SBUF so the DVE
    MAX8/FIND_INDEX8 run in 2x 16-bit mode.
  - csq folded into the PSUM drain: ACT casts PSUM->fp16 scratch, Pool adds
    the broadcast -(csq-768) row (fp16).  No 7th matmul per v-tile.
  - k-outer loop over v-groups of 4 so consecutive matmuls share the
    stationary x chunk; PSUM = one pool of 4 x [128,1024] (all 8 banks),
    the mix matmul allocates from the same pool.
  - top-2 exact rescore (fp32 gathers of candidate rows + fp32 x) exactly as
    the baseline, but the winner ROW is selected on the DVE from the two
    gathered candidates (flip*(g1-g0)+g0) instead of a 3rd gather.
  - noise pre-scaled to fp16 on host, added during the mix-PSUM drain on the
    DVE; output stored as fp16 and upcast on host.
"""

import os
import sys

for p in ("/opt/trn_rl_repo",):
    if p not in sys.path:
        sys.path.insert(0, p)

import numpy as np

import concourse.bacc as bacc
import concourse.bass as bass
import concourse.mybir as mybir
import concourse.tile as tile
from concourse.bass_utils import run_bass_kernel_spmd

B, S, D, V, K = 16, 512, 768, 4096, 4
N_CORES = 8
SS = S // N_CORES            # 64 sequence positions per core
T = SS * B                   # 1024 tokens per core, t = s_local*16 + b
TT = T // 128                # 8 token tiles per core
KC = D // 128                # 6 contraction chunks
NV = V // 512                # 8 V-tiles
NKK = KC // 2                # cb tiles hold k-pairs
DP_EPSILON = 0.1
CSQ_CENTER = 768.0
DE = 776                     # padded cb_ext row: 768 cb + 1 csq + 7 pad
GW = 769                     # gathered row width (cb + csq)

F32 = mybir.dt.float32
F16 = mybir.dt.float16
U16 = mybir.dt.uint16
I32 = mybir.dt.int32

# csq folding strategy: "preload" = ACT writes -(csq-768) into PSUM before
# the accumulation group; "matmul" = fp16 hi/lo K=2 matmul per v-tile.
CSQ_MODE = os.environ.get("CMDP_CSQ", "matmul")

_CACHED = {}


def _build_nc():
    nc = bacc.Bacc("TRN2", target_bir_lowering=False, debug=False,
                   num_devices=N_CORES)

    xT_d = nc.dram_tensor("xT", [128, TT * KC * 128], F16, kind="ExternalInput")
    cbT_d = nc.dram_tensor("cbT", [128, NV * KC * 512], F16, kind="ExternalInput")
    csqL_d = nc.dram_tensor("csqL", [2, T], F16, kind="ExternalInput")
    csqR_d = nc.dram_tensor("csqR", [2, V], F16, kind="ExternalInput")
    iota_d = nc.dram_tensor("iota32", [128, 32], F32, kind="ExternalInput")
    w_d = nc.dram_tensor("w", [128, TT * 128], F16, kind="ExternalInput")
    nz_d = nc.dram_tensor("nz", [T, D], F16, kind="ExternalInput")
    xn_d = nc.dram_tensor("xn", [128, TT * D], F32, kind="ExternalInput")
    cbe_d = nc.dram_tensor("cbe", [V, DE], F32, kind="ExternalInput")
    cb16_d = nc.dram_tensor("cb16", [V, D], F16, kind="ExternalInput")
    out_d = nc.dram_tensor("out", [T, D], F16, kind="ExternalOutput")

    XTW = KC * 128           # xT columns per token tile
    VBW = KC * 512           # cbT columns per v-block
    PW = 1024                # psum pair width (2 v-tiles)

    with tile.TileContext(nc) as tc:
        with (
            tc.tile_pool(name="big", bufs=1) as big,
            tc.tile_pool(name="sc", bufs=4) as sc_pool,
            tc.tile_pool(name="work", bufs=5) as work,
            tc.tile_pool(name="gp", bufs=6) as gpool,
            tc.tile_pool(name="ypool", bufs=6) as ypool,
            tc.tile_pool(name="io", bufs=5) as io,
            tc.tile_pool(name="ps", bufs=4, space="PSUM") as ps_pool,
        ):
            # --- input streaming (sync HWDGE queue), earliest-need order ---
            xt_t = []
            tl = big.tile([128, XTW], F16, tag="xt0")
            nc.sync.dma_start(tl[:], xT_d.ap()[:, 0:XTW])
            xt_t.append(tl)
            # cb tiles per (v, k-pair), v-major to match k-inner consumption
            cb_t = [[None] * NKK for _ in range(NV)]
            csql = big.tile([2, T], F16, tag="csql")
            nc.sync.dma_start(csql[:], csqL_d.ap())
            csqr = big.tile([2, V], F16, tag="csqr")
            nc.sync.dma_start(csqr[:], csqR_d.ap())
            iota32 = big.tile([128, 32], F32, tag="iota32")
            nc.sync.dma_start(iota32[:], iota_d.ap())
            for v in range(NV):
                for kk in range(NKK):
                    t2 = big.tile([128, 1024], F16, tag=f"cb{v}k{kk}")
                    nc.sync.dma_start(
                        t2[:],
                        cbT_d.ap()[:, v * VBW + kk * 1024:
                                   v * VBW + (kk + 1) * 1024])
                    cb_t[v][kk] = t2
            w = big.tile([128, TT * 128], F16, tag="w")
            nc.sync.dma_start(w[:], w_d.ap())
            xn_t = []
            for t in range(TT):
                if t >= 1:
                    tl = big.tile([128, XTW], F16, tag=f"xt{t}")
                    nc.sync.dma_start(
                        tl[:], xT_d.ap()[:, t * XTW:(t + 1) * XTW])
                    xt_t.append(tl)
                tl = big.tile([128, D], F32, tag=f"xn{t}")
                nc.sync.dma_start(tl[:], xn_d.ap()[:, t * D:(t + 1) * D])
                xn_t.append(tl)

            def emit_scoring(t, last):
                """fp16 scoring matmuls (k-inner per v-tile) + ACT drain per
                pair; incremental per-pair DVE scan only for the last tile
                (short tail) -- earlier tiles use a one-shot scan that
                overlaps the next tile's scoring."""
                tsl = slice(t * 128, (t + 1) * 128)
                scores = sc_pool.tile([128, V], F16, tag="scores")
                mxc = work.tile([128, 32], F16, tag="mxc")
                idxc = work.tile([128, 32], F32, tag="idxc")
                for g in range(2):
                    pairs = [ps_pool.tile([128, PW], F32, tag="ps",
                                          name=f"ps{pi}")
                             for pi in range(2)]
                    for vi in range(4):
                        v = 4 * g + vi
                        ps = pairs[vi // 2]
                        csl = slice((vi % 2) * 512, (vi % 2) * 512 + 512)
                        vsl = slice(v * 512, (v + 1) * 512)
                        nc.tensor.matmul(ps[:, csl], csql[:, tsl],
                                         csqr[:, vsl], start=True, stop=False)
                        for k in range(KC):
                            nc.tensor.matmul(
                                ps[:, csl],
                                xt_t[t][:, k * 128:(k + 1) * 128],
                                cb_t[v][k // 2][:, (k % 2) * 512:
                                                (k % 2) * 512 + 512],
                                start=False, stop=(k == KC - 1))
                        if vi % 2 == 1:
                            p = 2 * g + vi // 2
                            psl = slice((p * 2) * 512, (p * 2 + 2) * 512)
                            esl = slice(p * 8, (p + 1) * 8)
                            nc.scalar.copy(out=scores[:, psl],
                                           in_=pairs[vi // 2][:])
                            if last:
                                pidx = work.tile([128, 8], U16, tag="pidx")
                                nc.vector.max(mxc[:, esl], scores[:, psl])
                                nc.vector.max_index(pidx[:], mxc[:, esl],
                                                    scores[:, psl])
                                nc.vector.tensor_scalar(
                                    out=idxc[:, esl], in0=pidx[:],
                                    scalar1=float(p * PW), scalar2=None,
                                    op0=mybir.AluOpType.add)
                return scores, mxc, idxc

            def emit_scan_fixup(t, last, scores, mxc, idxc):
                """global top-2 (one-shot scan, or pair-merge for the last
                tile), exact fp32 rescore, winner row via fp16 gather."""
                xn = xn_t[t][:]
                cand = []
                if not last:
                    mx = work.tile([128, 8], F16, tag="mx")
                    idx = work.tile([128, 8], U16, tag="idx")
                    nc.vector.max(mx[:], scores[:])
                    nc.vector.max_index(idx[:], mx[:], scores[:])
                    for j in range(2):
                        cj = work.tile([128, 1], I32, tag=f"cand{j}")
                        nc.vector.tensor_copy(cj[:], idx[:, j:j + 1])
                        cand.append(cj)
                else:
                    mxf = work.tile([128, 8], F16, tag="mxf")
                    pos = work.tile([128, 8], U16, tag="pos")
                    nc.vector.max(mxf[:], mxc[:])
                    nc.vector.max_index(pos[:], mxf[:], mxc[:])
                    posf = work.tile([128, 8], F32, tag="posf")
                    nc.vector.tensor_copy(posf[:], pos[:])
                    for j in range(2):
                        mask = work.tile([128, 32], F32, tag=f"mask{j}")
                        nc.vector.tensor_scalar(
                            out=mask[:], in0=iota32[:],
                            scalar1=posf[:, j:j + 1],
                            scalar2=None, op0=mybir.AluOpType.is_equal)
                        mtmp = work.tile([128, 32], F32, tag=f"mtmp{j}")
                        cf = work.tile([128, 1], F32, tag=f"cf{j}")
                        nc.vector.scalar_tensor_tensor(
                            out=mtmp[:], in0=idxc[:], scalar=1.0, in1=mask[:],
                            op0=mybir.AluOpType.bypass,
                            op1=mybir.AluOpType.mult, accum_out=cf[:])
                        cj = work.tile([128, 1], I32, tag=f"cand{j}")
                        nc.vector.tensor_copy(cj[:], cf[:])
                        cand.append(cj)
                g = []
                sj = []
                for j in range(2):
                    gj = gpool.tile([128, DE], F32, tag=f"g{j}")
                    nc.gpsimd.indirect_dma_start(
                        out=gj[:], out_offset=None, in_=cbe_d.ap(),
                        in_offset=bass.IndirectOffsetOnAxis(
                            ap=cand[j][:, :1], axis=0))
                    g.append(gj)
                    # NB: tensor_tensor_reduce hard-faults TRN2 here;
                    # scalar_tensor_tensor with accum_out does not.
                    tmp = work.tile([128, D], F32, tag="rescore_tmp")
                    dj = work.tile([128, 1], F32, tag=f"d{j}")
                    nc.vector.scalar_tensor_tensor(
                        out=tmp[:], in0=xn, scalar=1.0, in1=gj[:, 0:D],
                        op0=mybir.AluOpType.bypass,
                        op1=mybir.AluOpType.mult, accum_out=dj[:])
                    s = work.tile([128, 1], F32, tag=f"s{j}")
                    # s = (dj * -2) + csq_cand
                    nc.vector.scalar_tensor_tensor(
                        out=s[:], in0=dj[:], scalar=-2.0, in1=gj[:, D:D + 1],
                        op0=mybir.AluOpType.mult, op1=mybir.AluOpType.add)
                    sj.append(s)
                flip = work.tile([128, 1], I32, tag="flip")
                nc.vector.tensor_tensor(out=flip[:], in0=sj[1][:],
                                        in1=sj[0][:],
                                        op=mybir.AluOpType.is_lt)
                # winner index = flip ? cand1 : cand0, then gather the fp16
                # codebook row directly (feeds the fp16 mix matmul)
                widx = work.tile([128, 1], I32, tag="widx")
                nc.vector.tensor_copy(widx[:], cand[0][:])
                nc.vector.copy_predicated(widx[:], flip[:], cand[1][:])
                y = ypool.tile([128, D], F16, tag="y")
                nc.gpsimd.indirect_dma_start(
                    out=y[:], out_offset=None, in_=cb16_d.ap(),
                    in_offset=bass.IndirectOffsetOnAxis(
                        ap=widx[:, :1], axis=0))
                return y

            def emit_output(t, y):
                """mix matmul -> ACT drain -> noise accum-DMA -> fp16 store."""
                tsl = slice(t * 128, (t + 1) * 128)
                pm = ps_pool.tile([128, PW], F32, tag="ps", name="pm")
                nc.tensor.matmul(pm[:, 0:512], w[:, tsl], y[:, 0:512],
                                 start=True, stop=True)
                nc.tensor.matmul(pm[:, 512:D], w[:, tsl], y[:, 512:D],
                                 start=True, stop=True)
                ob = io.tile([128, D], F16, tag="out")
                nc.scalar.copy(out=ob[:], in_=pm[:, 0:D])
                # add DP noise inline in the DMA (SWDGE accumulate, fp16)
                nc.gpsimd.dma_start(out=ob[:], in_=nz_d.ap()[tsl, :],
                                    accum_op=mybir.AluOpType.add)
                nc.sync.dma_start(out_d.ap()[tsl, :], ob[:])

            # software pipeline: scan/fixup of tile t overlaps scoring of
            # t+1..t+PIPE so the PE never waits on the gather/rescore chain.
            PIPE = 2
            pending = []
            for t in range(TT):
                last = (t == TT - 1)
                scores, mxc, idxc = emit_scoring(t, last)
                y = emit_scan_fixup(t, last, scores, mxc, idxc)
                pending.append((t, y))
                if len(pending) > PIPE:
                    emit_output(*pending.pop(0))
            for item in pending:
                emit_output(*item)

    nc.compile()
    return nc


def _prep_inputs(base_embeddings, codebook, rand_idx, noise):
    """Build the 8 per-core input maps (all host-side numpy)."""
    x = np.ascontiguousarray(base_embeddings, dtype=np.float32)
    cb = np.ascontiguousarray(codebook, dtype=np.float32)
    ridx = np.asarray(rand_idx)
    nz = np.asarray(noise, dtype=np.float32)

    csq = (cb * cb).sum(-1, dtype=np.float32)              # [V]
    cbe = np.zeros((V, DE), np.float32)
    cbe[:, :D] = cb
    cbe[:, D] = csq
    # broadcast -(csq-768) row, added to the 2x.c dots during the drain
    csqc = (csq - CSQ_CENTER).astype(np.float32)
    r1 = csqc.astype(np.float16)
    r2 = (csqc - r1.astype(np.float32)).astype(np.float16)
    csqR = np.ascontiguousarray(np.stack([r1, r2]))        # [2, V] fp16
    csqL = np.full((2, T), -1.0, np.float16)

    # pre-tile [D, V] -> [128, (v, k, 512)] v-block-major layout, fp16
    cbT = cb.T.reshape(KC, 128, NV, 512).transpose(1, 2, 0, 3).reshape(128, KC * V)
    cbT = np.ascontiguousarray(cbT.astype(np.float16))

    cb16 = np.ascontiguousarray(cb.astype(np.float16))
    iota32 = np.ascontiguousarray(
        np.broadcast_to(np.arange(32, dtype=np.float32)[None, :], (128, 32)))
    shared = {"cbe": cbe, "cbT": cbT, "csqL": csqL, "csqR": csqR,
              "cb16": cb16, "iota32": iota32}

    in_maps = []
    for c in range(N_CORES):
        ssl = slice(c * SS, (c + 1) * SS)
        # tokens t = s_local*16 + b
        xc = x[:, ssl, :].transpose(1, 0, 2).reshape(T, D)
        xT2 = (2.0 * xc).T                                 # [D, T]
        # pre-tile [D, T] -> [128, (t, k, 128)] tile-major layout, fp16
        xT2 = np.ascontiguousarray(
            xT2.reshape(KC, 128, TT, 128).transpose(1, 2, 0, 3)
            .reshape(128, KC * T).astype(np.float16))
        nzc = np.ascontiguousarray(
            (DP_EPSILON * nz[:, ssl, :].transpose(1, 0, 2).reshape(T, D))
            .astype(np.float16))
        rc = ridx[:, ssl, :]                               # [B, SS, K]
        wm = np.zeros((TT, 128, 128), np.float32)
        for tt in range(TT):
            for gi in range(8):
                s_local = tt * 8 + gi
                r = rc[:, s_local, :]                      # [B, K] in [0,B)
                cnt = np.zeros((B, B), np.float32)         # [dst=b, src]
                for bdst in range(B):
                    np.add.at(cnt[bdst], r[bdst], 1.0)
                wm[tt, gi * 16:(gi + 1) * 16, gi * 16:(gi + 1) * 16] = cnt.T / K
        wm_t = np.ascontiguousarray(
            wm.transpose(1, 0, 2).reshape(128, TT * 128).astype(np.float16))
        xnc = np.ascontiguousarray(
            xc.reshape(TT, 128, D).transpose(1, 0, 2).reshape(128, TT * D))
        m = {"xT": xT2, "w": wm_t, "nz": nzc, "xn": xnc, **shared}
        in_maps.append(m)
    return in_maps


def kernel(base_embeddings, codebook, rand_idx, noise, _results_out=None):
    if "nc" not in _CACHED:
        _CACHED["nc"] = _build_nc()
    nc = _CACHED["nc"]
    in_maps = _prep_inputs(base_embeddings, codebook, rand_idx, noise)
    res = run_bass_kernel_spmd(nc, in_maps, list(range(N_CORES)))
    if _results_out is not None:
        _results_out.append(res)
    outs = []
    for c in range(N_CORES):
        oc = res.results[c]["out"].astype(np.float32)
        oc = oc.reshape(SS, B, D).transpose(1, 0, 2)
        outs.append(oc)
    return np.ascontiguousarray(np.concatenate(outs, axis=1))


# revision 26
# speedup vs baseline: 1.0610x; 1.0610x over previous
"""Trainium2 Bass kernel for nn_CMDPEncoder (VQ codebook quantize + random
batch-mix dequantize + DP noise).

Reference semantics:
    dots = einsum('bsd,vd->bsv', base, codebook)
    qi   = argmin_v(csq[v] - 2*dots)                  # [B,S]
    codes[b,s,j] = qi[rand_idx[b,s,j], s]
    out  = mean_j codebook[codes] + 0.1*noise

Sharding: split the sequence dim S across the 8 cores (64 positions each).
The rand_idx mixing crosses only the batch dim at fixed s, so with S-sharding
every core's mixing is fully local (no collectives).  Tokens are laid out
s-major (t = s_local*16 + b) so each 128-token tile holds 8 complete
s-groups of 16 batches, and the mix becomes a block-diagonal [128,128]
matmul with host-precomputed weights (counts/4 from rand_idx).

v2 design (vs the fp32/bf16 baseline):
  - All scoring in fp16: x and codebook streamed as fp16 (same 1 cyc/row PE
    speed as bf16, 8x the mantissa), scores kept as fp16 in SBUF so the DVE
    MAX8/FIND_INDEX8 run in 2x 16-bit mode.
  - csq folded into the PSUM drain: ACT casts PSUM->fp16 scratch, Pool adds
    the broadcast -(csq-768) row (fp16).  No 7th matmul per v-tile.
  - k-outer loop over v-groups of 4 so consecutive matmuls share the
    stationary x chunk; PSUM = one pool of 4 x [128,1024] (all 8 banks),
    the mix matmul allocates from the same pool.
  - top-2 exact rescore (fp32 gathers of candidate rows + fp32 x) exactly as
    the baseline, but the winner ROW is selected on the DVE from the two
    gathered candidates (flip*(g1-g0)+g0) instead of a 3rd gather.
  - noise pre-scaled to fp16 on host, added during the mix-PSUM drain on the
    DVE; output stored as fp16 and upcast on host.
"""

import os
import sys

for p in ("/opt/trn_rl_repo",):
    if p not in sys.path:
        sys.path.insert(0, p)

import numpy as np

import concourse.bacc as bacc
import concourse.bass as bass
import concourse.mybir as mybir
import concourse.tile as tile
from concourse.bass_utils import run_bass_kernel_spmd

B, S, D, V, K = 16, 512, 768, 4096, 4
N_CORES = 8
SS = S // N_CORES            # 64 sequence positions per core
T = SS * B                   # 1024 tokens per core, t = s_local*16 + b
TT = T // 128                # 8 token tiles per core
KC = D // 128                # 6 contraction chunks
NV = V // 512                # 8 V-tiles
NKK = KC // 2                # cb tiles hold k-pairs
DP_EPSILON = 0.1
CSQ_CENTER = 768.0
DE = 776                     # padded cb_ext row: 768 cb + 1 csq + 7 pad
GW = 769                     # gathered row width (cb + csq)

F32 = mybir.dt.float32
F16 = mybir.dt.float16
U16 = mybir.dt.uint16
I32 = mybir.dt.int32

# csq folding strategy: "preload" = ACT writes -(csq-768) into PSUM before
# the accumulation group; "matmul" = fp16 hi/lo K=2 matmul per v-tile.
CSQ_MODE = os.environ.get("CMDP_CSQ", "matmul")

_CACHED = {}


def _build_nc():
    nc = bacc.Bacc("TRN2", target_bir_lowering=False, debug=False,
                   num_devices=N_CORES)

    xT_d = nc.dram_tensor("xT", [128, TT * KC * 128], F16, kind="ExternalInput")
    cbT_d = nc.dram_tensor("cbT", [128, NV * KC * 512], F16, kind="ExternalInput")
    csqL_d = nc.dram_tensor("csqL", [2, T], F16, kind="ExternalInput")
    csqR_d = nc.dram_tensor("csqR", [2, V], F16, kind="ExternalInput")
    iota_d = nc.dram_tensor("iota32", [128, 32], F32, kind="ExternalInput")
    w_d = nc.dram_tensor("w", [128, TT * 128], F16, kind="ExternalInput")
    nz_d = nc.dram_tensor("nz", [T, D], F16, kind="ExternalInput")
    xn_d = nc.dram_tensor("xn", [128, TT * D], F32, kind="ExternalInput")
    cbe_d = nc.dram_tensor("cbe", [V, DE], F32, kind="ExternalInput")
    cb16_d = nc.dram_tensor("cb16", [V, D], F16, kind="ExternalInput")
    out_d = nc.dram_tensor("out", [T, D], F16, kind="ExternalOutput")

    XTW = KC * 128           # xT columns per token tile
    VBW = KC * 512           # cbT columns per v-block
    PW = 1024                # psum pair width (2 v-tiles)

    with tile.TileContext(nc) as tc:
        with (
            tc.tile_pool(name="big", bufs=1) as big,
            tc.tile_pool(name="sc", bufs=4) as sc_pool,
            tc.tile_pool(name="work", bufs=5) as work,
            tc.tile_pool(name="gp", bufs=6) as gpool,
            tc.tile_pool(name="ypool", bufs=6) as ypool,
            tc.tile_pool(name="io", bufs=5) as io,
            tc.tile_pool(name="ps", bufs=3, space="PSUM") as ps_pool,
            tc.tile_pool(name="pm", bufs=1, space="PSUM") as pm_pool,
        ):
            # --- input streaming (sync HWDGE queue), earliest-need order ---
            xt_t = []
            tl = big.tile([128, XTW], F16, tag="xt0")
            nc.sync.dma_start(tl[:], xT_d.ap()[:, 0:XTW])
            xt_t.append(tl)
            # cb tiles per (v, k-pair), v-major to match k-inner consumption
            cb_t = [[None] * NKK for _ in range(NV)]
            csql = big.tile([2, T], F16, tag="csql")
            nc.sync.dma_start(csql[:], csqL_d.ap())
            csqr = big.tile([2, V], F16, tag="csqr")
            nc.sync.dma_start(csqr[:], csqR_d.ap())
            iota32 = big.tile([128, 32], F32, tag="iota32")
            nc.sync.dma_start(iota32[:], iota_d.ap())
            for v in range(NV):
                for kk in range(NKK):
                    t2 = big.tile([128, 1024], F16, tag=f"cb{v}k{kk}")
                    nc.sync.dma_start(
                        t2[:],
                        cbT_d.ap()[:, v * VBW + kk * 1024:
                                   v * VBW + (kk + 1) * 1024])
                    cb_t[v][kk] = t2
            w = big.tile([128, TT * 128], F16, tag="w")
            nc.sync.dma_start(w[:], w_d.ap())
            xn_t = []
            for t in range(TT):
                if t >= 1:
                    tl = big.tile([128, XTW], F16, tag=f"xt{t}")
                    nc.sync.dma_start(
                        tl[:], xT_d.ap()[:, t * XTW:(t + 1) * XTW])
                    xt_t.append(tl)
                tl = big.tile([128, D], F32, tag=f"xn{t}")
                nc.sync.dma_start(tl[:], xn_d.ap()[:, t * D:(t + 1) * D])
                xn_t.append(tl)

            def emit_scoring(t, last):
                """fp16 scoring matmuls (k-inner per v-tile) + ACT drain per
                pair; incremental per-pair DVE scan only for the last tile
                (short tail) -- earlier tiles use a one-shot scan that
                overlaps the next tile's scoring."""
                tsl = slice(t * 128, (t + 1) * 128)
                scores = sc_pool.tile([128, V], F16, tag="scores")
                mxc = work.tile([128, 32], F16, tag="mxc")
                idxc = work.tile([128, 32], F32, tag="idxc")
                for g in range(2):
                    pairs = [ps_pool.tile([128, PW], F32, tag="ps",
                                          name=f"ps{pi}")
                             for pi in range(2)]
                    for vi in range(4):
                        v = 4 * g + vi
                        ps = pairs[vi // 2]
                        csl = slice((vi % 2) * 512, (vi % 2) * 512 + 512)
                        vsl = slice(v * 512, (v + 1) * 512)
                        nc.tensor.matmul(ps[:, csl], csql[:, tsl],
                                         csqr[:, vsl], start=True, stop=False)
                        for k in range(KC):
                            nc.tensor.matmul(
                                ps[:, csl],
                                xt_t[t][:, k * 128:(k + 1) * 128],
                                cb_t[v][k // 2][:, (k % 2) * 512:
                                                (k % 2) * 512 + 512],
                                start=False, stop=(k == KC - 1))
                        if vi % 2 == 1:
                            p = 2 * g + vi // 2
                            psl = slice((p * 2) * 512, (p * 2 + 2) * 512)
                            esl = slice(p * 8, (p + 1) * 8)
                            nc.scalar.copy(out=scores[:, psl],
                                           in_=pairs[vi // 2][:])
                            if last:
                                pidx = work.tile([128, 8], U16, tag="pidx")
                                nc.vector.max(mxc[:, esl], scores[:, psl])
                                nc.vector.max_index(pidx[:], mxc[:, esl],
                                                    scores[:, psl])
                                nc.vector.tensor_scalar(
                                    out=idxc[:, esl], in0=pidx[:],
                                    scalar1=float(p * PW), scalar2=None,
                                    op0=mybir.AluOpType.add)
                return scores, mxc, idxc

            def emit_scan_fixup(t, last, scores, mxc, idxc):
                """global top-2 (one-shot scan, or pair-merge for the last
                tile), exact fp32 rescore, winner row via fp16 gather."""
                xn = xn_t[t][:]
                cand = []
                if not last:
                    mx = work.tile([128, 8], F16, tag="mx")
                    idx = work.tile([128, 8], U16, tag="idx")
                    nc.vector.max(mx[:], scores[:])
                    nc.vector.max_index(idx[:], mx[:], scores[:])
                    for j in range(2):
                        cj = work.tile([128, 1], I32, tag=f"cand{j}")
                        nc.vector.tensor_copy(cj[:], idx[:, j:j + 1])
                        cand.append(cj)
                else:
                    mxf = work.tile([128, 8], F16, tag="mxf")
                    pos = work.tile([128, 8], U16, tag="pos")
                    nc.vector.max(mxf[:], mxc[:])
                    nc.vector.max_index(pos[:], mxf[:], mxc[:])
                    posf = work.tile([128, 8], F32, tag="posf")
                    nc.vector.tensor_copy(posf[:], pos[:])
                    for j in range(2):
                        mask = work.tile([128, 32], F32, tag=f"mask{j}")
                        nc.vector.tensor_scalar(
                            out=mask[:], in0=iota32[:],
                            scalar1=posf[:, j:j + 1],
                            scalar2=None, op0=mybir.AluOpType.is_equal)
                        mtmp = work.tile([128, 32], F32, tag=f"mtmp{j}")
                        cf = work.tile([128, 1], F32, tag=f"cf{j}")
                        nc.vector.scalar_tensor_tensor(
                            out=mtmp[:], in0=idxc[:], scalar=1.0, in1=mask[:],
                            op0=mybir.AluOpType.bypass,
                            op1=mybir.AluOpType.mult, accum_out=cf[:])
                        cj = work.tile([128, 1], I32, tag=f"cand{j}")
                        nc.vector.tensor_copy(cj[:], cf[:])
                        cand.append(cj)
                g = []
                sj = []
                for j in range(2):
                    gj = gpool.tile([128, DE], F32, tag=f"g{j}")
                    nc.gpsimd.indirect_dma_start(
                        out=gj[:], out_offset=None, in_=cbe_d.ap(),
                        in_offset=bass.IndirectOffsetOnAxis(
                            ap=cand[j][:, :1], axis=0))
                    g.append(gj)
                    # NB: tensor_tensor_reduce hard-faults TRN2 here;
                    # scalar_tensor_tensor with accum_out does not.
                    tmp = work.tile([128, D], F32, tag="rescore_tmp")
                    dj = work.tile([128, 1], F32, tag=f"d{j}")
                    nc.vector.scalar_tensor_tensor(
                        out=tmp[:], in0=xn, scalar=1.0, in1=gj[:, 0:D],
                        op0=mybir.AluOpType.bypass,
                        op1=mybir.AluOpType.mult, accum_out=dj[:])
                    s = work.tile([128, 1], F32, tag=f"s{j}")
                    # s = (dj * -2) + csq_cand
                    nc.vector.scalar_tensor_tensor(
                        out=s[:], in0=dj[:], scalar=-2.0, in1=gj[:, D:D + 1],
                        op0=mybir.AluOpType.mult, op1=mybir.AluOpType.add)
                    sj.append(s)
                flip = work.tile([128, 1], I32, tag="flip")
                nc.vector.tensor_tensor(out=flip[:], in0=sj[1][:],
                                        in1=sj[0][:],
                                        op=mybir.AluOpType.is_lt)
                # winner index = flip ? cand1 : cand0, then gather the fp16
                # codebook row directly (feeds the fp16 mix matmul)
                widx = work.tile([128, 1], I32, tag="widx")
                nc.vector.tensor_copy(widx[:], cand[0][:])
                nc.vector.copy_predicated(widx[:], flip[:], cand[1][:])
                y = ypool.tile([128, D], F16, tag="y")
                nc.gpsimd.indirect_dma_start(
                    out=y[:], out_offset=None, in_=cb16_d.ap(),
                    in_offset=bass.IndirectOffsetOnAxis(
                        ap=widx[:, :1], axis=0))
                return y

            def emit_output(t, y):
                """mix matmul -> ACT drain -> noise accum-DMA -> fp16 store."""
                tsl = slice(t * 128, (t + 1) * 128)
                pm = pm_pool.tile([128, PW], F32, tag="pm", name="pm")
                nc.tensor.matmul(pm[:, 0:512], w[:, tsl], y[:, 0:512],
                                 start=True, stop=True)
                nc.tensor.matmul(pm[:, 512:D], w[:, tsl], y[:, 512:D],
                                 start=True, stop=True)
                ob = io.tile([128, D], F16, tag="out")
                nc.scalar.copy(out=ob[:], in_=pm[:, 0:D])
                # add DP noise inline in the DMA (SWDGE accumulate, fp16)
                nc.gpsimd.dma_start(out=ob[:], in_=nz_d.ap()[tsl, :],
                                    accum_op=mybir.AluOpType.add)
                nc.sync.dma_start(out_d.ap()[tsl, :], ob[:])

            # software pipeline: scan/fixup of tile t overlaps scoring of
            # t+1..t+PIPE so the PE never waits on the gather/rescore chain.
            PIPE = 2
            pending = []
            for t in range(TT):
                last = (t >= TT - 2)
                scores, mxc, idxc = emit_scoring(t, last)
                y = emit_scan_fixup(t, last, scores, mxc, idxc)
                pending.append((t, y))
                if len(pending) > PIPE:
                    emit_output(*pending.pop(0))
            for item in pending:
                emit_output(*item)

    nc.compile()
    return nc


def _prep_inputs(base_embeddings, codebook, rand_idx, noise):
    """Build the 8 per-core input maps (all host-side numpy)."""
    x = np.ascontiguousarray(base_embeddings, dtype=np.float32)
    cb = np.ascontiguousarray(codebook, dtype=np.float32)
    ridx = np.asarray(rand_idx)
    nz = np.asarray(noise, dtype=np.float32)

    csq = (cb * cb).sum(-1, dtype=np.float32)              # [V]
    cbe = np.zeros((V, DE), np.float32)
    cbe[:, :D] = cb
    cbe[:, D] = csq
    # broadcast -(csq-768) row, added to the 2x.c dots during the drain
    csqc = (csq - CSQ_CENTER).astype(np.float32)
    r1 = csqc.astype(np.float16)
    r2 = (csqc - r1.astype(np.float32)).astype(np.float16)
    csqR = np.ascontiguousarray(np.stack([r1, r2]))        # [2, V] fp16
    csqL = np.full((2, T), -1.0, np.float16)

    # pre-tile [D, V] -> [128, (v, k, 512)] v-block-major layout, fp16
    cbT = cb.T.reshape(KC, 128, NV, 512).transpose(1, 2, 0, 3).reshape(128, KC * V)
    cbT = np.ascontiguousarray(cbT.astype(np.float16))

    cb16 = np.ascontiguousarray(cb.astype(np.float16))
    iota32 = np.ascontiguousarray(
        np.broadcast_to(np.arange(32, dtype=np.float32)[None, :], (128, 32)))
    shared = {"cbe": cbe, "cbT": cbT, "csqL": csqL, "csqR": csqR,
              "cb16": cb16, "iota32": iota32}

    in_maps = []
    for c in range(N_CORES):
        ssl = slice(c * SS, (c + 1) * SS)
        # tokens t = s_local*16 + b
        xc = x[:, ssl, :].transpose(1, 0, 2).reshape(T, D)
        xT2 = (2.0 * xc).T                                 # [D, T]
        # pre-tile [D, T] -> [128, (t, k, 128)] tile-major layout, fp16
        xT2 = np.ascontiguousarray(
            xT2.reshape(KC, 128, TT, 128).transpose(1, 2, 0, 3)
            .reshape(128, KC * T).astype(np.float16))
        nzc = np.ascontiguousarray(
            (DP_EPSILON * nz[:, ssl, :].transpose(1, 0, 2).reshape(T, D))
            .astype(np.float16))
        rc = ridx[:, ssl, :]                               # [B, SS, K]
        wm = np.zeros((TT, 128, 128), np.float32)
        for tt in range(TT):
            for gi in range(8):
                s_local = tt * 8 + gi
                r = rc[:, s_local, :]                      # [B, K] in [0,B)
                cnt = np.zeros((B, B), np.float32)         # [dst=b, src]
                for bdst in range(B):
                    np.add.at(cnt[bdst], r[bdst], 1.0)
                wm[tt, gi * 16:(gi + 1) * 16, gi * 16:(gi + 1) * 16] = cnt.T / K
        wm_t = np.ascontiguousarray(
            wm.transpose(1, 0, 2).reshape(128, TT * 128).astype(np.float16))
        xnc = np.ascontiguousarray(
            xc.reshape(TT, 128, D).transpose(1, 0, 2).reshape(128, TT * D))
        m = {"xT": xT2, "w": wm_t, "nz": nzc, "xn": xnc, **shared}
        in_maps.append(m)
    return in_maps


def kernel(base_embeddings, codebook, rand_idx, noise, _results_out=None):
    if "nc" not in _CACHED:
        _CACHED["nc"] = _build_nc()
    nc = _CACHED["nc"]
    in_maps = _prep_inputs(base_embeddings, codebook, rand_idx, noise)
    res = run_bass_kernel_spmd(nc, in_maps, list(range(N_CORES)))
    if _results_out is not None:
        _results_out.append(res)
    outs = []
    for c in range(N_CORES):
        oc = res.results[c]["out"].astype(np.float32)
        oc = oc.reshape(SS, B, D).transpose(1, 0, 2)
        outs.append(oc)
    return np.ascontiguousarray(np.concatenate(outs, axis=1))
